# revision 63
# baseline (speedup 1.0000x reference)
"""Delta-modulator scan kernel for Trainium2 (Bass/Tile).

Problem: x [128, 1024, 252] f32. Per (b, r): sequential scan over the first
232 columns with state (dc, delta, trig/quiet run counters); outputs
UP[232] | DN[232] | x[:, :, 232:252]  ->  out [128, 1024, 484] f32.

Sharding: pure data parallel over batch (16 batches / core, 8 cores).
Per-core layout: 16384 instances = [128 partitions x 128 free]; the scan
runs as 232 vectorized steps over [128, 128] state tiles.

State encoding:
  dc    : last accepted sample (f32)
  dl    : delta in {0.02, 0.1} exactly
  cc    : signed run counter (c>0: c consecutive trigs; c<0: -c consecutive quiets)
Update per step t (exact wrt reference):
  y    = x_t - dc
  up   = y > dl                      -> output
  dn   = (-y) > dl                   -> output
  trig = up + dn
  dc   = trig ? x_t : dc             (copy_predicated)
  cp   = max(cc, 0) + 1
  cc   = min(cc, 0) - 1
  cc   = trig ? cp : cc              (copy_predicated)
  A    = (cc <= -3) * 0.1
  u    = max(A, dl)
  cap  = max((cc < 3), 0.02)         ((cc<3) in {0,1}; 1.0 acts as +inf vs delta)
  dl   = min(u, cap)
"""

import os
from contextlib import ExitStack

import numpy as np

import concourse.bass as bass
import concourse.tile as tile
from concourse import bacc, mybir
from concourse.bass_utils import run_bass_kernel_spmd
import concourse.dve_ops as dve_ops_mod
from concourse.dve_spec import (
    Spec, Src0, Src1, C0, C1, C2, Zero, One, maxx, minn, select, lower,
)
from concourse.dve_spec import _has_src1
from concourse.dve_uop import DveOpSpec

AluOp = mybir.AluOpType
F32 = mybir.dt.float32


def _register_op(name: str, spec: Spec) -> "dve_ops_mod.DveOp":
    """Register a custom DVE op at runtime (compute + pin its uop sha)."""
    for existing in dve_ops_mod.OPS:
        if existing.name == name:
            return existing
    opcode = dve_ops_mod._CUSTOM_DVE_ROW_BASE + len(dve_ops_mod.OPS)
    assert opcode < 0x20
    shas = {}
    for ver in ("v3",):
        tmp = DveOpSpec(
            name=name, opcode=opcode, uops=lower(spec, ver=ver), rd1_en=_has_src1(spec)
        )
        shas[ver] = tmp.sha(ver)
    op = dve_ops_mod.DveOp(name, spec, subdim=False, uops_sha=shas)
    dve_ops_mod.OPS.append(op)
    dve_ops_mod._SUB_OPCODE_FOR_NAME[name] = opcode
    dve_ops_mod.CUSTOM_DVE_SPECS[name] = spec
    return op


# cc' = trig ? max(cc,0)+1 : min(cc,0)-1   (in0=cc, in1=trig in {0.0,1.0})
DM_COUNTER = _register_op(
    "DM_COUNTER_ANT",
    Spec(
        body=select(Src1, maxx(Src0, Zero) + One, minn(Src0, Zero) - One),
        reference=lambda in0, in1, s0, s1, imm2: np.where(
            in1 != 0.0, np.maximum(in0, 0) + 1, np.minimum(in0, 0) - 1
        ).astype(np.float32),
    ),
)

# trig = |y| > dl   (in0=y, in1=dl)
DM_TRIG = _register_op(
    "DM_TRIG_ANT",
    Spec(
        body=maxx(Src0, Zero - Src0) > Src1,
        reference=lambda in0, in1, s0, s1, imm2: (
            np.abs(in0) > in1
        ).astype(np.float32),
    ),
)

# dl' = min(max(dl, (cc<=-3)*0.1), max((cc<3), 0.02))  (in0=cc, in1=dl,
# s0=-3.0, s1=0.1, imm2=0.02)
DM_DELTA = _register_op(
    "DM_DELTA_ANT",
    Spec(
        body=minn(
            maxx(Src1, (Src0 <= C0) * C1),
            maxx(Src0 < (Zero - C0), C2),
        ),
        reference=lambda in0, in1, s0, s1, imm2: np.minimum(
            np.maximum(in1, (in0 <= s0).astype(np.float32) * s1),
            np.maximum((in0 < -s0).astype(np.float32), imm2),
        ).astype(np.float32),
    ),
)

# v = (y > dl) - (y < -dl)  in {-1, 0, +1} (never -0.0): +1 = up-trigger,
# -1 = down-trigger, 0 = no trigger. Doubles as the predication mask
# (bit pattern nonzero iff trigger).  (in0=y, in1=dl)
DM_V = _register_op(
    "DM_V2_ANT",
    Spec(
        body=(Src0 > Src1) - (Src0 < (Zero - Src1)),
        reference=lambda in0, in1, s0, s1, imm2: (
            (in0 > in1).astype(np.float32) - (in0 < -in1).astype(np.float32)
        ),
    ),
)

B, R, C = 128, 1024, 252
NSTEP = 232
NTAIL = C - NSTEP  # 20
OUTC = 2 * NSTEP + NTAIL  # 484
NCORES = 8
BPC = B // NCORES  # 16
INST = BPC * R  # 16384 instances per core
P = 128
F = INST // P  # 128

_NC_CACHE = {}


def _kernel_body(tc: "tile.TileContext", out: bass.AP, x: bass.AP) -> None:
    nc = tc.nc
    x3 = x.rearrange("(p f) c -> p f c", p=P)  # [128, 128, 252]
    o3 = out.rearrange("(p f) c -> p f c", p=P)  # [128, 128, 484]

    PASSA = 128  # pass A covers cols [0, 128); pass B covers [128, 232)
    with ExitStack() as ctx:
        state = ctx.enter_context(tc.tile_pool(name="state", bufs=1))
        xpool = ctx.enter_context(tc.tile_pool(name="xp", bufs=1))
        opool = ctx.enter_context(tc.tile_pool(name="op", bufs=1))
        tmp = ctx.enter_context(tc.tile_pool(name="tmp", bufs=6))

        dc = state.tile([P, F], F32, tag="dc")
        dl = state.tile([P, F], F32, tag="dl0")
        cc = state.tile([P, F], F32, tag="cc0")
        nc.vector.memset(dc[:], 0.0)
        nc.vector.memset(dl[:], 0.1)
        nc.vector.memset(cc[:], 0.0)
        tg = 0

        Sign = mybir.ActivationFunctionType.Sign
        Relu = mybir.ActivationFunctionType.Relu

        def step(xs, up, dn):
            nonlocal dc, dl, cc, tg
            y = tmp.tile([P, F], F32, tag="y")
            nc.gpsimd.tensor_tensor(y[:], xs, dc[:], AluOp.subtract)
            v = tmp.tile([P, F], F32, tag="v")
            nc.vector._custom_dve(DM_V, out=v[:], in0=y[:], in1=dl[:])
            nc.vector.copy_predicated(dc[:], v[:].bitcast(mybir.dt.int32), xs)
            cc2 = state.tile([P, F], F32, tag=f"cc{(tg + 1) % 2}")
            nc.vector._custom_dve(DM_COUNTER, out=cc2[:], in0=cc[:], in1=v[:])
            dl2 = state.tile([P, F], F32, tag=f"dl{(tg + 1) % 2}")
            nc.vector._custom_dve(
                DM_DELTA, out=dl2[:], in0=cc2[:], in1=dl[:],
                s0=-3.0, s1=0.1, imm2=0.02,
            )
            nc.scalar.activation(up, v[:], Relu, 0.0, 1.0)
            nc.scalar.activation(dn, v[:], Relu, 0.0, -1.0)
            cc, dl = cc2, dl2
            tg += 1

        # ---- pass A: cols [0, PASSA) ----
        xt = xpool.tile([P, F, PASSA], F32, tag="xt")
        for k0, kn in ((0, 8), (8, 8), (16, 16), (32, 32), (64, 32), (96, 32)):
            nc.sync.dma_start(xt[:, :, k0 : k0 + kn], x3[:, :, k0 : k0 + kn])
        upt = opool.tile([P, F, PASSA], F32, tag="upt")
        dnt = opool.tile([P, F, PASSA], F32, tag="dnt")

        NB = NSTEP - PASSA  # 104
        B0 = C - PASSA  # 124
        OFF = PASSA - B0  # 4
        IN_CH = 32
        # pass-B tiles share slots with pass-A tiles (same tag, bufs=1);
        # loads are emitted inside pass A's loop so the SP queue reaches
        # them early — Tile's range-level WAR deps keep it correct.
        xt2 = xpool.tile([P, F, PASSA], F32, tag="xt")
        upt2 = opool.tile([P, F, NB], F32, tag="upt")
        dnt2 = opool.tile([P, F, NB], F32, tag="dnt")

        QD = 32
        for t in range(PASSA):
            step(xt[:, :, t], upt[:, :, t], dnt[:, :, t])
            if t % QD == 15 and t > QD:
                q0 = (t // QD - 1) * QD
                nc.sync.dma_start(
                    o3[:, :, q0 : q0 + QD], upt[:, :, q0 : q0 + QD]
                )
                nc.sync.dma_start(
                    o3[:, :, NSTEP + q0 : NSTEP + q0 + QD],
                    dnt[:, :, q0 : q0 + QD],
                )
        q0 = PASSA - QD
        nc.sync.dma_start(xt2[:, :, 0:IN_CH], x3[:, :, B0 : B0 + IN_CH])
        nc.sync.dma_start(o3[:, :, q0:PASSA], upt[:, :, q0:PASSA])
        nc.sync.dma_start(
            o3[:, :, NSTEP + q0 : NSTEP + PASSA], dnt[:, :, q0:PASSA]
        )
        for k in range(IN_CH, PASSA, IN_CH):
            nc.sync.dma_start(
                xt2[:, :, k : k + IN_CH], x3[:, :, B0 + k : B0 + k + IN_CH]
            )

        # ---- pass B: cols [PASSA, NSTEP) ----
        for t in range(NB):
            step(xt2[:, :, t + OFF], upt2[:, :, t], dnt2[:, :, t])
            if t % QD == 15 and QD < t < 3 * QD:
                q0 = (t // QD - 1) * QD
                nc.sync.dma_start(
                    o3[:, :, PASSA + q0 : PASSA + q0 + QD],
                    upt2[:, :, q0 : q0 + QD],
                )
                nc.sync.dma_start(
                    o3[:, :, NSTEP + PASSA + q0 : NSTEP + PASSA + q0 + QD],
                    dnt2[:, :, q0 : q0 + QD],
                )
            if t in (80, 96):
                # trailing drains in 16-col pieces as soon as they complete
                q0 = t - 16
                nc.sync.dma_start(
                    o3[:, :, PASSA + q0 : PASSA + t], upt2[:, :, q0:t]
                )
                nc.sync.dma_start(
                    o3[:, :, NSTEP + PASSA + q0 : NSTEP + PASSA + t],
                    dnt2[:, :, q0:t],
                )
        nc.sync.dma_start(o3[:, :, PASSA + 96 : NSTEP], upt2[:, :, 96:NB])
        nc.sync.dma_start(
            o3[:, :, NSTEP + PASSA + 96 : 2 * NSTEP], dnt2[:, :, 96:NB]
        )
        # tail passthrough from the pass-B input tile (cols [232, 252))
        nc.sync.dma_start(
            o3[:, :, 2 * NSTEP : OUTC], xt2[:, :, NSTEP - B0 : PASSA]
        )


def _build_nc() -> bass.Bass:
    key = "nc"
    if key in _NC_CACHE:
        return _NC_CACHE[key]
    nc = bacc.Bacc("TRN2", target_bir_lowering=False, debug=False)
    x = nc.dram_tensor("x", [INST, C], F32, kind="ExternalInput").ap()
    out = nc.dram_tensor("out", [INST, OUTC], F32, kind="ExternalOutput").ap()
    with tile.TileContext(nc) as tc:
        _kernel_body(tc, out, x)
    nc.compile()
    _NC_CACHE[key] = nc
    return nc


def kernel(x: np.ndarray) -> np.ndarray:
    x = np.ascontiguousarray(np.asarray(x), dtype=np.float32)
    assert x.shape == (B, R, C), x.shape
    nc = _build_nc()
    in_maps = [
        {"x": np.ascontiguousarray(x[c * BPC : (c + 1) * BPC].reshape(INST, C))}
        for c in range(NCORES)
    ]
    res = run_bass_kernel_spmd(
        nc,
        in_maps,
        core_ids=list(range(NCORES)),
        trace=bool(int(os.environ.get("KERNEL_TRACE", "0"))),
    )
    global LAST_RESULTS
    LAST_RESULTS = res
    outs = [r["out"].reshape(BPC, R, OUTC) for r in res.results]
    return np.concatenate(outs, axis=0)


LAST_RESULTS = None


if __name__ == "__main__":
    xs = np.random.default_rng(0).standard_normal((B, R, C), dtype=np.float32)
    o = kernel(xs)
    print(o.shape, o.dtype)
